# revision 20
# baseline (speedup 1.0000x reference)
"""Causal multi-head attention (B=2, T=2048, C=1024, H=16, d=64) on 8 trn2 cores.

Sharding: core i -> (batch b = i//4, head group g = i%4, 4 heads/core).
Data parallel over B, tensor parallel over heads; the out-proj partial sums
(contraction over this core's 256 channels) are reduced on the host during
the gather step, along with b_proj and the analytically-folded V bias.

All matmul operands are bfloat16 (full PE rate at any N; fp32 PSUM accum);
the y partials are DMA'd from PSUM in fp32 and summed on the host.

Device kernel works in [feature, token] (transposed) layout; ascending
q-blocks with stage-1 interleaved in chunks so the PE ramps with the DMA:
  stage 0: a few junk warmup matmuls on an uninitialized tile keep the PE
           HAM activity monitor busy during the initial DMA wait, so real
           matmuls start at the warm 2.4 GHz clock instead of 1.2 GHz
  stage 1 (chunk tc4): Q^T,K^T cols = (Wqk)^T x^T (+bias, 1/sqrt(d) folded
           into Wq host-side); V t-tiles 4*tc4..4*tc4+3 in natural layout.
           Chunk 0 upfront; chunk qb+1 emitted inside q-block qb's head loop.
  stage 2: S^T[j,q] = K_h^T.T @ Q_h^T  per head; the head pair's matmuls
           (K=64, row groups 0/64) are emitted back-to-back alternating row
           groups so the PE runs them CONCURRENTLY (2x row tiling)
  stage 3: P^T = exp(S^T); the causal mask is applied AFTER exp as a 0/1
           multiply on the 128-wide diagonal block (off the S->P critical
           path; hidden behind the O-matmul lag)
  stage 4: outT[128,q] = [V_h | 1s(64)]^T.T @ P^T accumulated over j tiles
           (rows 64:128 = softmax denominator Z replicated 64x); O-matmuls
           are emitted a few groups behind their exp so the in-order PE
           never waits on ACT
  stage 5: att^T = outT[0:64] * reciprocal(outT[64:128])  (pure DVE)
  stage 6: y^T = Wp.T @ att^T -> fp16 out; host sums.
"""

import numpy as np
import ml_dtypes

import concourse.bass as bass
import concourse.mybir as mybir
from concourse import bacc
import concourse.tile as tile
from concourse.bass_utils import run_bass_kernel_spmd

B, T, C, H, D = 2, 2048, 1024, 16, 64
NCORES = 8
HPC = 4            # heads per core
CS = HPC * D       # 256 channels per core (per Q/K/V block)
KT = C // 128      # 8 contraction tiles for the projections
NT = T // 128      # 16 token tiles of 128
QB = 512           # query block (psum bank width in fp32)
NQB = T // QB      # 4 query blocks

F32 = mybir.dt.float32
F16 = mybir.dt.float16
BF = mybir.dt.bfloat16
BF_NP = ml_dtypes.bfloat16

TRACE = False
LAST_RESULT = None


def _build_body(nc, tc, ctx, xT, wqk, wv, bqk, wp, masks, yT, yT2):
    AF = mybir.ActivationFunctionType

    persist = ctx.enter_context(tc.tile_pool(name="persist", bufs=1))

    wqk_sb = persist.tile([128, KT, 2 * CS], BF, tag="wqk", name="wqk")
    wv_sb = persist.tile([128, KT, CS], BF, tag="wv", name="wv")
    bqk_sb = persist.tile([128, 4], F32, tag="bqk", name="bqk")
    wp_sb = persist.tile([128, 2, C], BF, tag="wp", name="wp")
    mask_sb = persist.tile([128, 128], BF, tag="mask", name="mask_sb")
    warm_sb = persist.tile([128, QB], BF, tag="warm", name="warm_sb")
    qT_sb = [persist.tile([128, T], BF, tag=f"qT{i}", name=f"qT{i}") for i in range(2)]
    kT_sb = [persist.tile([128, T], BF, tag=f"kT{i}", name=f"kT{i}") for i in range(2)]
    # per j-tile: 4 heads x (64 ones cols | 64 V cols); the ones columns
    # replicate the softmax denominator into outT rows 0:64 (base partition 0
    # so reciprocal_approx_fast reads it correctly; custom DVE ops ignore a
    # nonzero input base partition)
    v_sb = [persist.tile([128, HPC, 2 * D], BF, tag=f"v{t}", name=f"v{t}") for t in range(NT)]
    attT_sb = [persist.tile([128, T], BF, tag=f"attT{i}", name=f"attT{i}") for i in range(2)]
    xT_sb = persist.tile([128, KT, T], BF, tag="xT", name="xT")

    widep = ctx.enter_context(tc.tile_pool(name="wide", bufs=2, space="PSUM"))
    oTp = ctx.enter_context(tc.tile_pool(name="oT", bufs=2, space="PSUM"))
    fillp = ctx.enter_context(tc.tile_pool(name="fill", bufs=1, space="PSUM"))
    pTp = ctx.enter_context(tc.tile_pool(name="pT", bufs=8))
    smallp = ctx.enter_context(tc.tile_pool(name="small", bufs=8))
    ysp = ctx.enter_context(tc.tile_pool(name="ystage", bufs=6))

    # ---------------- PE warmup ----------------
    # junk matmuls (no data deps beyond one tiny memset) issue right after the
    # framework preamble and keep the PE busy through the DMA ramp, so the HAM
    # un-throttles to 2.4 GHz before the first real matmul
    nc.gpsimd.memset(warm_sb[:, :], 0.0)

    def emit_warm(n, pool=fillp):
        wps = pool.tile([128, 2, QB], F32, tag="sT", name="warmps")
        for _ in range(n):
            nc.tensor.matmul(wps[:, 0, :], lhsT=warm_sb[:, 0:128],
                             rhs=warm_sb[:, :], start=True, stop=True)

    # ---------------- DMAs ----------------
    # each dma_start costs ~600ns of serial issue time on its engine, so the
    # per-k tiles are merged into wide transfers, issues are split across the
    # two HWDGE engines (SP + ACT), and the big x column-tail transfers are
    # issued from the gpsimd SWDGE queue BEFORE its memsets so the fabric
    # starts streaming them early
    xTr = xT.rearrange("(k p) t -> p k t", k=KT)
    wqkr = wqk.rearrange("(k p) c -> p k c", k=KT)
    # x head (cols 0:1024) as per-k 256KB pieces with 2KB lines (~40% better
    # HBM read efficiency than 1KB); each piece also carries chunk-1's data,
    # so chunk 1 never waits on the slow SWDGE queue
    for k in range(KT):
        nc.sync.dma_start(out=xT_sb[:, k:k + 1, 0:2 * QB],
                          in_=xTr[:, k:k + 1, 0:2 * QB])
    nc.scalar.dma_start(out=wqk_sb[:, 0:1, :], in_=wqkr[:, 0:1, :])
    nc.scalar.dma_start(out=wqk_sb[:, 1:2, :], in_=wqkr[:, 1:2, :])
    nc.scalar.dma_start(out=wqk_sb[:, 2:4, :], in_=wqkr[:, 2:4, :])
    nc.scalar.dma_start(out=wqk_sb[:, 4:KT, :], in_=wqkr[:, 4:KT, :])
    nc.scalar.dma_start(out=mask_sb[:, :], in_=masks[:, :])
    nc.scalar.dma_start(out=bqk_sb[:, :], in_=bqk.rearrange("(c p) one -> p (c one)", c=4))
    nc.scalar.dma_start(out=wv_sb[:, :, :], in_=wv.rearrange("(k p) c -> p k c", k=KT))
    for t in range(NT):
        nc.gpsimd.memset(v_sb[t][:, :, :], 1.0)
    nc.gpsimd.dma_start(out=wp_sb[:, :, :], in_=wp.rearrange("(k p) c -> p k c", k=2))
    # x column tail (cols 1024:2048) as per-k 256KB pieces with 2KB lines
    for k in range(KT):
        nc.gpsimd.dma_start(
            out=xT_sb[:, k:k + 1, 2 * QB:T],
            in_=xTr[:, k:k + 1, 2 * QB:T],
        )

    # ---------------- filler work units ----------------
    # stage-1 chunks and stage-6 out-proj are chopped into ~0.2-0.4us PE
    # units and drained between attention groups, filling every ACT-lag
    # bubble while keeping the in-order PE busy
    def qk_pair_units(tc4, pair, pool):
        ref = {}
        units = []

        def bias_add(m):
            ps = ref["ps"]
            ct = pair[m]
            dst = qT_sb[ct] if ct < 2 else kT_sb[ct - 2]
            nc.vector.tensor_scalar_add(
                dst[:, tc4 * QB:(tc4 + 1) * QB], ps[:, m, :], bqk_sb[:, ct:ct + 1]
            )

        def kstep(k):
            def u():
                if k == 0:
                    ref["ps"] = pool.tile([128, 2, QB], F32, tag="sT", name="wps")
                ps = ref["ps"]
                for m, ct in enumerate(pair):
                    nc.tensor.matmul(
                        ps[:, m, :],
                        lhsT=wqk_sb[:, k, ct * 128:(ct + 1) * 128],
                        rhs=xT_sb[:, k, tc4 * QB:(tc4 + 1) * QB],
                        start=(k == 0),
                        stop=(k == KT - 1),
                    )
                if k == KT - 1:
                    bias_add(0)   # m0 drain starts a unit early
            return u

        units = [kstep(k) for k in range(KT)]
        units.append(lambda: bias_add(1))
        return units

    def v_pair_units(tc4, tp, pool, tag="sT"):
        t0 = 4 * tc4 + 2 * tp
        ref = {}

        def copies(m):
            # one contiguous copy per head: strided/rearranged dst APs
            # mis-register their write deps (O-matmuls then race the copy)
            ps = ref["ps"]
            t = t0 + m
            for h in range(HPC):
                nc.vector.tensor_copy(
                    v_sb[t][:, h, D:2 * D], ps[:, m, h * D:(h + 1) * D]
                )

        def kstep(k):
            def u():
                if k == 0:
                    ref["ps"] = pool.tile([128, 2, QB], F32, tag=tag, name="wps")
                ps = ref["ps"]
                for m in range(2):
                    t = t0 + m
                    nc.tensor.matmul(
                        ps[:, m, 0:CS],
                        lhsT=xT_sb[:, k, t * 128:(t + 1) * 128],
                        rhs=wv_sb[:, k, :],
                        start=(k == 0),
                        stop=(k == KT - 1),
                    )
                if k == KT - 1:
                    copies(0)   # t0 drain starts a unit early
            return u

        units = [kstep(k) for k in range(KT)]
        units.append(lambda: copies(1))
        return units

    def chunk_units(tc4, pool):
        units = []
        for pair in ((0, 2), (1, 3)):
            units += qk_pair_units(tc4, pair, pool)
        for tp in range(2):
            units += v_pair_units(tc4, tp, pool)
        return units

    def stage6_units(sqb, pools, kcs=(0, 1), out_t=None, alt_copy=False,
                     alt_dma=False):
        # one combined unit per et: the CAST is emitted right after its chain
        # so it lands early in the in-order vector queue (no convoy behind
        # norm-muls); consecutive units are spaced >=2 groups by the fq drain
        ot = out_t if out_t is not None else yT
        units = []
        for et in range(C // 128):
            def u(et=et):
                pool = pools[et % len(pools)]
                yps_w = pool.tile([128, 2, QB], F32, tag="sT", name="yps")
                yps_t = yps_w[:, 0, :]
                for j, kc in enumerate(kcs):
                    nc.tensor.matmul(
                        yps_t,
                        lhsT=wp_sb[:, kc, et * 128:(et + 1) * 128],
                        rhs=attT_sb[kc][:, sqb * QB:(sqb + 1) * QB],
                        start=(j == 0),
                        stop=(j == len(kcs) - 1),
                    )
                ys = ysp.tile([128, QB], F16, tag="ys", name="ys")
                if alt_copy and et % 2 == 1:
                    nc.scalar.activation(ys[:, :], yps_t, AF.Copy)
                else:
                    nc.vector.tensor_copy(ys[:, :], yps_t)
                eng = nc.scalar if (alt_dma and et % 2 == 1) else nc.sync
                eng.dma_start(
                    out=ot[et * 128:(et + 1) * 128, sqb * QB:(sqb + 1) * QB],
                    in_=ys[:, :],
                )
            units.append(u)
        return units

    # ---------------- attention piece emitters ----------------
    def emit_S(h, qb, grp, sT):
        ktile = kT_sb[h // 2]
        qtile = qT_sb[h // 2]
        po = (h % 2) * D
        diag = grp >= 2 * (qb + 1) - 2
        for m in range(2):
            jt = grp * 2 + m
            c0 = 128 * (jt - 4 * qb) if diag else 0
            nc.tensor.matmul(
                sT[:, m, c0:QB],
                lhsT=ktile[po:po + D, jt * 128:(jt + 1) * 128],
                rhs=qtile[po:po + D, qb * QB + c0:(qb + 1) * QB],
                start=True,
                stop=True,
            )

    def emit_exp(h, qb, grp, sT, pT):
        diag = grp >= 2 * (qb + 1) - 2
        if diag:
            # ONE fused exp for both m j-tiles starting at m0's causal
            # offset; the m1 columns below its own offset become junk in pT
            # but the O-matmul only streams pT from its per-tile offset, so
            # they are never read. Saves the ~293ns fixed ACTIVATE overhead
            # of a second instruction on the pacing ACT engine.
            c0 = 128 * (grp * 2 - 4 * qb)
            nc.scalar.activation(pT[:, :, c0:QB], sT[:, :, c0:QB], AF.Exp)
            # 0/1 causal mask on the diagonal 128-block, after exp so the
            # S->P critical path has no DVE hop; hidden by the O lag
            for m in range(2):
                cm = 128 * (grp * 2 + m - 4 * qb)
                nc.vector.tensor_mul(
                    pT[:, m, cm:cm + 128], pT[:, m, cm:cm + 128], mask_sb[:, :]
                )
        else:
            nc.scalar.activation(pT[:, :, :], sT[:, :, :], AF.Exp)

    def emit_O(h, qb, grp, pT, oT):
        njt = 4 * (qb + 1)
        diag = grp >= 2 * (qb + 1) - 2
        for m in range(2):
            jt = grp * 2 + m
            c0 = 128 * (jt - 4 * qb) if diag else 0
            nc.tensor.matmul(
                oT[:, c0:QB],
                lhsT=v_sb[jt][:, h, :],
                rhs=pT[:, m, c0:QB],
                start=(jt == 0),
                stop=(jt == njt - 1),
            )

    def emit_norm(h, qb, oT):
        po = (h % 2) * D
        rz = smallp.tile([D, QB], F32, tag="rz", name="rz")
        nc.vector.reciprocal_approx_fast(out=rz[:, :], in_=oT[0:D, :])
        nc.vector.tensor_mul(
            attT_sb[h // 2][po:po + D, qb * QB:(qb + 1) * QB],
            oT[D:2 * D, :],
            rz[:, :],
        )

    # ---------------- main schedule ----------------
    # chunk 0 upfront (the DMA ramp); spread its 4 psum tiles over both pools
    for un in qk_pair_units(0, (0, 2), widep):
        un()
    for un in qk_pair_units(0, (1, 3), widep):
        un()
    for un in v_pair_units(0, 0, fillp):
        un()
    for un in v_pair_units(0, 1, fillp):
        un()

    dq = []   # deadline units: chunk(qb+1) must finish within qb
    fq = []   # free units: stage6, drained evenly over ALL remaining groups

    total_groups = HPC * sum(2 * (qb + 1) for qb in range(NQB))
    state = {"tg_left": total_groups}

    def drain(qb_groups_left):
        n = -(-len(dq) // qb_groups_left) if qb_groups_left > 0 else len(dq)
        for _ in range(min(n, len(dq))):
            dq.pop(0)()
        nf = -(-len(fq) // state["tg_left"]) if state["tg_left"] > 0 else len(fq)
        for _ in range(min(nf, len(fq))):
            fq.pop(0)()

    for qb in range(NQB):
        ngr = 2 * (qb + 1)
        if qb >= 1:
            fq.extend(stage6_units(qb - 1, [fillp]))
        if qb < NQB - 1:
            dq.extend(chunk_units(qb + 1, fillp))
        g_left = HPC * ngr
        # heads run in pairs, groups round-robin across the pair: exp(h,g)
        # gets two heads' worth of PE time before its O-matmuls are due, so
        # the single ACT stream never gates the in-order PE
        for hp in ((0, 1), (2, 3)):
            oTs = {h: oTp.tile([128, QB], F32, tag="oT", name="oT") for h in hp}
            pend = []
            for grp in range(ngr):
                for h in hp:
                    sT = widep.tile([128, 2, QB], F32, tag="sT", name="sT")
                    emit_S(h, qb, grp, sT)
                    pT = pTp.tile([128, 2, QB], BF, tag="pT", name="pT")
                    emit_exp(h, qb, grp, sT, pT)
                    drain(g_left)
                    g_left -= 1
                    state["tg_left"] -= 1
                    pend.append((h, grp, pT))
                    if len(pend) > 4:
                        h0_, g0, p0 = pend.pop(0)
                        emit_O(h0_, qb, g0, p0, oTs[h0_])
            if dq:
                dq.pop(0)()
            elif fq:
                fq.pop(0)()
            for h0_, g0, p0 in pend:
                emit_O(h0_, qb, g0, p0, oTs[h0_])
            for h in hp:
                emit_norm(h, qb, oTs[h])
            if qb == NQB - 1 and hp == (0, 1):
                # last q-block: its kc0 out-proj half only needs heads 0,1 —
                # feed it to the tail-end groups as free filler
                fq.extend(stage6_units(NQB - 1, [fillp], kcs=(0,)))
        while dq:
            dq.pop(0)()
    while fq:
        fq.pop(0)()
    # tail: kc1 half of the last q-block -> yT2 (host-summed). A few junk
    # matmuls bridge the PE-idle norm window (keeps the HAM clock warm), then
    # the 8 units pipeline over two psum pools with alternating copy engines
    # and alternating DMA queues
    emit_warm(6, pool=widep)
    for un in stage6_units(NQB - 1, [fillp, widep], kcs=(1,), out_t=yT2,
                           alt_copy=True, alt_dma=True):
        un()


def build_nc():
    from contextlib import ExitStack

    nc = bacc.Bacc("TRN2", target_bir_lowering=False)
    xT = nc.dram_tensor("xT", [C, T], BF, kind="ExternalInput")
    wqk = nc.dram_tensor("wqk", [C, 2 * CS], BF, kind="ExternalInput")
    wv = nc.dram_tensor("wv", [C, CS], BF, kind="ExternalInput")
    bqk = nc.dram_tensor("bqk", [2 * CS, 1], F32, kind="ExternalInput")
    wp = nc.dram_tensor("wp", [CS, C], BF, kind="ExternalInput")
    masks = nc.dram_tensor("masks", [128, 128], BF, kind="ExternalInput")
    yT = nc.dram_tensor("yT", [C, T], F16, kind="ExternalOutput")
    yT2 = nc.dram_tensor("yT2", [C, T], F16, kind="ExternalOutput")
    with tile.TileContext(nc) as tc:
        with nc.allow_low_precision(reason="bf16 matmul inputs; accumulation stays fp32 in PSUM"):
            with ExitStack() as ctx:
                _build_body(nc, tc, ctx, xT, wqk, wv, bqk, wp, masks, yT, yT2)
    nc.compile()
    return nc


def make_mask01():
    r = np.arange(128)[:, None]
    c = np.arange(128)[None, :]
    return np.where(r <= c, 1.0, 0.0).astype(BF_NP)


def make_in_maps(x, W_qkv, b_qkv, W_proj):
    scale = np.float32(1.0 / np.sqrt(D))
    mask_h = make_mask01()
    in_maps = []
    for i in range(NCORES):
        b, g = divmod(i, HPC)
        cs0 = g * CS
        wq = W_qkv[:, cs0:cs0 + CS] * scale
        wk = W_qkv[:, C + cs0:C + cs0 + CS]
        bq = b_qkv[cs0:cs0 + CS] * scale
        bk = b_qkv[C + cs0:C + cs0 + CS]
        in_maps.append({
            "xT": np.ascontiguousarray(x[b].T).astype(BF_NP),
            "wqk": np.concatenate([wq, wk], axis=1).astype(BF_NP),
            "wv": np.ascontiguousarray(W_qkv[:, 2 * C + cs0:2 * C + cs0 + CS]).astype(BF_NP),
            "bqk": np.concatenate([bq, bk])[:, None].astype(np.float32),
            "wp": np.ascontiguousarray(W_proj[cs0:cs0 + CS, :]).astype(BF_NP),
            "masks": mask_h,
        })
    return in_maps


_NC_CACHE = None


def _get_nc():
    global _NC_CACHE
    if _NC_CACHE is None:
        _NC_CACHE = build_nc()
    return _NC_CACHE


def gather(results, b_qkv, W_proj, b_proj):
    Y = np.zeros((B, T, C), np.float32)
    for i in range(NCORES):
        Y[i // HPC] += results[i]["yT"].T.astype(np.float32)
        qb3 = (NQB - 1) * QB
        Y[i // HPC][qb3:] += results[i]["yT2"].T[qb3:].astype(np.float32)
    Y += (b_qkv[2 * C:].astype(np.float32) @ W_proj.astype(np.float32)
          + b_proj.astype(np.float32))[None, None, :]
    return Y


def kernel(x, W_qkv, b_qkv, W_proj, b_proj):
    global LAST_RESULT
    x = np.asarray(x, np.float32)
    W_qkv = np.asarray(W_qkv, np.float32)
    b_qkv = np.asarray(b_qkv, np.float32)
    W_proj = np.asarray(W_proj, np.float32)
    b_proj = np.asarray(b_proj, np.float32)

    nc = _get_nc()
    in_maps = make_in_maps(x, W_qkv, b_qkv, W_proj)
    res = run_bass_kernel_spmd(nc, in_maps, list(range(NCORES)), trace=TRACE)
    LAST_RESULT = res
    if TRACE and res.exec_time_ns is not None:
        print(f"HW exec time: {res.exec_time_ns} ns")
    return gather(res.results, b_qkv, W_proj, b_proj)


# revision 22
# speedup vs baseline: 1.0460x; 1.0460x over previous
"""Causal multi-head attention (B=2, T=2048, C=1024, H=16, d=64) on 8 trn2 cores.

Sharding: core i -> (batch b = i//4, head group g = i%4, 4 heads/core).
Data parallel over B, tensor parallel over heads; the out-proj partial sums
(contraction over this core's 256 channels) are reduced on the host during
the gather step, along with b_proj and the analytically-folded V bias.

All matmul operands are bfloat16 (full PE rate at any N; fp32 PSUM accum);
the y partials are DMA'd from PSUM in fp32 and summed on the host.

Device kernel works in [feature, token] (transposed) layout; ascending
q-blocks with stage-1 interleaved in chunks so the PE ramps with the DMA:
  stage 0: a few junk warmup matmuls on an uninitialized tile keep the PE
           HAM activity monitor busy during the initial DMA wait, so real
           matmuls start at the warm 2.4 GHz clock instead of 1.2 GHz
  stage 1 (chunk tc4): Q^T,K^T cols = (Wqk)^T x^T (+bias, 1/sqrt(d) folded
           into Wq host-side); V t-tiles 4*tc4..4*tc4+3 in natural layout.
           Chunk 0 upfront; chunk qb+1 emitted inside q-block qb's head loop.
  stage 2: S^T[j,q] = K_h^T.T @ Q_h^T  per head; the head pair's matmuls
           (K=64, row groups 0/64) are emitted back-to-back alternating row
           groups so the PE runs them CONCURRENTLY (2x row tiling)
  stage 3: P^T = exp(S^T); the causal mask is applied AFTER exp as a 0/1
           multiply on the 128-wide diagonal block (off the S->P critical
           path; hidden behind the O-matmul lag)
  stage 4: outT[128,q] = [V_h | 1s(64)]^T.T @ P^T accumulated over j tiles
           (rows 64:128 = softmax denominator Z replicated 64x); O-matmuls
           are emitted a few groups behind their exp so the in-order PE
           never waits on ACT
  stage 5: att^T = outT[0:64] * reciprocal(outT[64:128])  (pure DVE)
  stage 6: y^T = Wp.T @ att^T -> fp16 out; host sums.
"""

import numpy as np
import ml_dtypes

import concourse.bass as bass
import concourse.mybir as mybir
from concourse import bacc
import concourse.tile as tile
from concourse.bass_utils import run_bass_kernel_spmd

B, T, C, H, D = 2, 2048, 1024, 16, 64
NCORES = 8
HPC = 4            # heads per core
CS = HPC * D       # 256 channels per core (per Q/K/V block)
KT = C // 128      # 8 contraction tiles for the projections
NT = T // 128      # 16 token tiles of 128
QB = 512           # query block (psum bank width in fp32)
NQB = T // QB      # 4 query blocks

F32 = mybir.dt.float32
F16 = mybir.dt.float16
BF = mybir.dt.bfloat16
BF_NP = ml_dtypes.bfloat16

TRACE = False
LAST_RESULT = None


def _build_body(nc, tc, ctx, xT, wqk, wv, bqk, wp, masks, yT, yT2):
    AF = mybir.ActivationFunctionType

    persist = ctx.enter_context(tc.tile_pool(name="persist", bufs=1))

    wqk_sb = persist.tile([128, KT, 2 * CS], BF, tag="wqk", name="wqk")
    wv_sb = persist.tile([128, KT, CS], BF, tag="wv", name="wv")
    bqk_sb = persist.tile([128, 4], F32, tag="bqk", name="bqk")
    wp_sb = persist.tile([128, 2, C], BF, tag="wp", name="wp")
    mask_sb = persist.tile([128, 128], BF, tag="mask", name="mask_sb")
    warm_sb = persist.tile([128, QB], BF, tag="warm", name="warm_sb")
    qT_sb = [persist.tile([128, T], BF, tag=f"qT{i}", name=f"qT{i}") for i in range(2)]
    kT_sb = [persist.tile([128, T], BF, tag=f"kT{i}", name=f"kT{i}") for i in range(2)]
    # per j-tile: 4 heads x (64 ones cols | 64 V cols); the ones columns
    # replicate the softmax denominator into outT rows 0:64 (base partition 0
    # so reciprocal_approx_fast reads it correctly; custom DVE ops ignore a
    # nonzero input base partition)
    v_sb = [persist.tile([128, HPC, 2 * D], BF, tag=f"v{t}", name=f"v{t}") for t in range(NT)]
    attT_sb = [persist.tile([128, T], BF, tag=f"attT{i}", name=f"attT{i}") for i in range(2)]
    xT_sb = persist.tile([128, KT, T], BF, tag="xT", name="xT")

    widep = ctx.enter_context(tc.tile_pool(name="wide", bufs=2, space="PSUM"))
    oTp = ctx.enter_context(tc.tile_pool(name="oT", bufs=2, space="PSUM"))
    fillp = ctx.enter_context(tc.tile_pool(name="fill", bufs=1, space="PSUM"))
    pTp = ctx.enter_context(tc.tile_pool(name="pT", bufs=8))
    smallp = ctx.enter_context(tc.tile_pool(name="small", bufs=8))
    ysp = ctx.enter_context(tc.tile_pool(name="ystage", bufs=6))

    # ---------------- PE warmup ----------------
    # junk matmuls (no data deps beyond one tiny memset) issue right after the
    # framework preamble and keep the PE busy through the DMA ramp, so the HAM
    # un-throttles to 2.4 GHz before the first real matmul
    nc.gpsimd.memset(warm_sb[:, :], 0.0)

    def emit_warm(n, pool=fillp):
        wps = pool.tile([128, 2, QB], F32, tag="sT", name="warmps")
        for _ in range(n):
            nc.tensor.matmul(wps[:, 0, :], lhsT=warm_sb[:, 0:128],
                             rhs=warm_sb[:, :], start=True, stop=True)

    # ---------------- DMAs ----------------
    # each dma_start costs ~600ns of serial issue time on its engine, so the
    # per-k tiles are merged into wide transfers, issues are split across the
    # two HWDGE engines (SP + ACT), and the big x column-tail transfers are
    # issued from the gpsimd SWDGE queue BEFORE its memsets so the fabric
    # starts streaming them early
    xTr = xT.rearrange("(k p) t -> p k t", k=KT)
    wqkr = wqk.rearrange("(k p) c -> p k c", k=KT)
    nc.sync.dma_start(out=xT_sb[:, 0:1, 0:QB], in_=xTr[:, 0:1, 0:QB])
    nc.scalar.dma_start(out=wqk_sb[:, 0:1, :], in_=wqkr[:, 0:1, :])
    nc.sync.dma_start(out=xT_sb[:, 1:2, 0:QB], in_=xTr[:, 1:2, 0:QB])
    nc.scalar.dma_start(out=wqk_sb[:, 1:2, :], in_=wqkr[:, 1:2, :])
    nc.sync.dma_start(out=xT_sb[:, 2:4, 0:QB], in_=xTr[:, 2:4, 0:QB])
    nc.scalar.dma_start(out=wqk_sb[:, 2:4, :], in_=wqkr[:, 2:4, :])
    nc.sync.dma_start(out=xT_sb[:, 4:KT, 0:QB], in_=xTr[:, 4:KT, 0:QB])
    nc.scalar.dma_start(out=wqk_sb[:, 4:KT, :], in_=wqkr[:, 4:KT, :])
    nc.scalar.dma_start(out=mask_sb[:, :], in_=masks[:, :])
    nc.scalar.dma_start(out=bqk_sb[:, :], in_=bqk.rearrange("(c p) one -> p (c one)", c=4))
    nc.scalar.dma_start(out=wv_sb[:, :, :], in_=wv.rearrange("(k p) c -> p k c", k=KT))
    for t in range(NT):
        nc.gpsimd.memset(v_sb[t][:, :, :], 1.0)
    nc.gpsimd.dma_start(out=wp_sb[:, :, :], in_=wp.rearrange("(k p) c -> p k c", k=2))
    # x column tail (cols 512:2048) as per-k 384KB pieces with 3KB lines:
    # each chunk-unit kstep waits only its own k piece, and pieces arrive
    # k-ascending in consumption order
    for k in range(KT):
        nc.gpsimd.dma_start(
            out=xT_sb[:, k:k + 1, QB:T],
            in_=xTr[:, k:k + 1, QB:T],
        )

    # ---------------- filler work units ----------------
    # stage-1 chunks and stage-6 out-proj are chopped into ~0.2-0.4us PE
    # units and drained between attention groups, filling every ACT-lag
    # bubble while keeping the in-order PE busy
    def qk_pair_units(tc4, pair, pool):
        ref = {}
        units = []

        def bias_add(m):
            ps = ref["ps"]
            ct = pair[m]
            dst = qT_sb[ct] if ct < 2 else kT_sb[ct - 2]
            nc.vector.tensor_scalar_add(
                dst[:, tc4 * QB:(tc4 + 1) * QB], ps[:, m, :], bqk_sb[:, ct:ct + 1]
            )

        def kstep(k):
            def u():
                if k == 0:
                    ref["ps"] = pool.tile([128, 2, QB], F32, tag="sT", name="wps")
                ps = ref["ps"]
                for m, ct in enumerate(pair):
                    nc.tensor.matmul(
                        ps[:, m, :],
                        lhsT=wqk_sb[:, k, ct * 128:(ct + 1) * 128],
                        rhs=xT_sb[:, k, tc4 * QB:(tc4 + 1) * QB],
                        start=(k == 0),
                        stop=(k == KT - 1),
                    )
                if k == KT - 1:
                    bias_add(0)   # m0 drain starts a unit early
            return u

        units = [kstep(k) for k in range(KT)]
        units.append(lambda: bias_add(1))
        return units

    def v_pair_units(tc4, tp, pool, tag="sT"):
        t0 = 4 * tc4 + 2 * tp
        ref = {}

        def copies(m):
            # one contiguous copy per head: strided/rearranged dst APs
            # mis-register their write deps (O-matmuls then race the copy)
            ps = ref["ps"]
            t = t0 + m
            for h in range(HPC):
                nc.vector.tensor_copy(
                    v_sb[t][:, h, D:2 * D], ps[:, m, h * D:(h + 1) * D]
                )

        def kstep(k):
            def u():
                if k == 0:
                    ref["ps"] = pool.tile([128, 2, QB], F32, tag=tag, name="wps")
                ps = ref["ps"]
                for m in range(2):
                    t = t0 + m
                    nc.tensor.matmul(
                        ps[:, m, 0:CS],
                        lhsT=xT_sb[:, k, t * 128:(t + 1) * 128],
                        rhs=wv_sb[:, k, :],
                        start=(k == 0),
                        stop=(k == KT - 1),
                    )
                if k == KT - 1:
                    copies(0)   # t0 drain starts a unit early
            return u

        units = [kstep(k) for k in range(KT)]
        units.append(lambda: copies(1))
        return units

    def chunk_units(tc4, pool):
        units = []
        for pair in ((0, 2), (1, 3)):
            units += qk_pair_units(tc4, pair, pool)
        for tp in range(2):
            units += v_pair_units(tc4, tp, pool)
        return units

    def stage6_units(sqb, pools, kcs=(0, 1), out_t=None, alt_copy=False,
                     alt_dma=False):
        # one combined unit per et: the CAST is emitted right after its chain
        # so it lands early in the in-order vector queue (no convoy behind
        # norm-muls); consecutive units are spaced >=2 groups by the fq drain
        ot = out_t if out_t is not None else yT
        units = []
        for et in range(C // 128):
            def u(et=et):
                pool = pools[et % len(pools)]
                yps_w = pool.tile([128, 2, QB], F32, tag="sT", name="yps")
                yps_t = yps_w[:, 0, :]
                for j, kc in enumerate(kcs):
                    nc.tensor.matmul(
                        yps_t,
                        lhsT=wp_sb[:, kc, et * 128:(et + 1) * 128],
                        rhs=attT_sb[kc][:, sqb * QB:(sqb + 1) * QB],
                        start=(j == 0),
                        stop=(j == len(kcs) - 1),
                    )
                ys = ysp.tile([128, QB], F16, tag="ys", name="ys")
                if alt_copy and et % 2 == 1:
                    nc.scalar.activation(ys[:, :], yps_t, AF.Copy)
                else:
                    nc.vector.tensor_copy(ys[:, :], yps_t)
                eng = nc.scalar if (alt_dma and et % 2 == 1) else nc.sync
                eng.dma_start(
                    out=ot[et * 128:(et + 1) * 128, sqb * QB:(sqb + 1) * QB],
                    in_=ys[:, :],
                )
            units.append(u)
        return units

    # ---------------- attention piece emitters ----------------
    def emit_S(h, qb, grp, sT):
        ktile = kT_sb[h // 2]
        qtile = qT_sb[h // 2]
        po = (h % 2) * D
        diag = grp >= 2 * (qb + 1) - 2
        # high_priority pins the two K=64 matmuls back-to-back on the PE:
        # both become ready at once (same psum tile WAR release), and without
        # the pin the scheduler slots a full-array O/filler matmul between
        # them, paying the 64-row <-> 128-row tiling mode switch twice
        with tc.high_priority():
            for m in range(2):
                jt = grp * 2 + m
                c0 = 128 * (jt - 4 * qb) if diag else 0
                nc.tensor.matmul(
                    sT[:, m, c0:QB],
                    lhsT=ktile[po:po + D, jt * 128:(jt + 1) * 128],
                    rhs=qtile[po:po + D, qb * QB + c0:(qb + 1) * QB],
                    start=True,
                    stop=True,
                )

    def emit_exp(h, qb, grp, sT, pT):
        diag = grp >= 2 * (qb + 1) - 2
        if diag:
            # ONE fused exp for both m j-tiles starting at m0's causal
            # offset; the m1 columns below its own offset become junk in pT
            # but the O-matmul only streams pT from its per-tile offset, so
            # they are never read. Saves the ~293ns fixed ACTIVATE overhead
            # of a second instruction on the pacing ACT engine.
            c0 = 128 * (grp * 2 - 4 * qb)
            nc.scalar.activation(pT[:, :, c0:QB], sT[:, :, c0:QB], AF.Exp)
            # 0/1 causal mask on the diagonal 128-block, after exp so the
            # S->P critical path has no DVE hop; hidden by the O lag
            for m in range(2):
                cm = 128 * (grp * 2 + m - 4 * qb)
                nc.vector.tensor_mul(
                    pT[:, m, cm:cm + 128], pT[:, m, cm:cm + 128], mask_sb[:, :]
                )
        else:
            nc.scalar.activation(pT[:, :, :], sT[:, :, :], AF.Exp)

    def emit_O(h, qb, grp, pT, oT):
        njt = 4 * (qb + 1)
        diag = grp >= 2 * (qb + 1) - 2
        for m in range(2):
            jt = grp * 2 + m
            c0 = 128 * (jt - 4 * qb) if diag else 0
            nc.tensor.matmul(
                oT[:, c0:QB],
                lhsT=v_sb[jt][:, h, :],
                rhs=pT[:, m, c0:QB],
                start=(jt == 0),
                stop=(jt == njt - 1),
            )

    def emit_norm(h, qb, oT):
        po = (h % 2) * D
        rz = smallp.tile([D, QB], F32, tag="rz", name="rz")
        nc.vector.reciprocal_approx_fast(out=rz[:, :], in_=oT[0:D, :])
        nc.vector.tensor_mul(
            attT_sb[h // 2][po:po + D, qb * QB:(qb + 1) * QB],
            oT[D:2 * D, :],
            rz[:, :],
        )

    # ---------------- main schedule ----------------
    # chunk 0 upfront (the DMA ramp); spread its 4 psum tiles over both pools
    for un in qk_pair_units(0, (0, 2), widep):
        un()
    for un in qk_pair_units(0, (1, 3), widep):
        un()
    for un in v_pair_units(0, 0, fillp):
        un()
    for un in v_pair_units(0, 1, fillp):
        un()

    dq = []   # deadline units: chunk(qb+1) must finish within qb
    fq = []   # free units: stage6, drained evenly over ALL remaining groups

    total_groups = HPC * sum(2 * (qb + 1) for qb in range(NQB))
    state = {"tg_left": total_groups}

    def drain(qb_groups_left):
        n = -(-len(dq) // qb_groups_left) if qb_groups_left > 0 else len(dq)
        for _ in range(min(n, len(dq))):
            dq.pop(0)()
        nf = -(-len(fq) // state["tg_left"]) if state["tg_left"] > 0 else len(fq)
        for _ in range(min(nf, len(fq))):
            fq.pop(0)()

    for qb in range(NQB):
        ngr = 2 * (qb + 1)
        if qb >= 1:
            fq.extend(stage6_units(qb - 1, [fillp]))
        if qb < NQB - 1:
            dq.extend(chunk_units(qb + 1, fillp))
        g_left = HPC * ngr
        # heads run in pairs, groups round-robin across the pair: exp(h,g)
        # gets two heads' worth of PE time before its O-matmuls are due, so
        # the single ACT stream never gates the in-order PE
        for hp in ((0, 1), (2, 3)):
            oTs = {h: oTp.tile([128, QB], F32, tag="oT", name="oT") for h in hp}
            pend = []
            for grp in range(ngr):
                for h in hp:
                    sT = widep.tile([128, 2, QB], F32, tag="sT", name="sT")
                    emit_S(h, qb, grp, sT)
                    pT = pTp.tile([128, 2, QB], BF, tag="pT", name="pT")
                    emit_exp(h, qb, grp, sT, pT)
                    drain(g_left)
                    g_left -= 1
                    state["tg_left"] -= 1
                    pend.append((h, grp, pT))
                    if len(pend) > 4:
                        h0_, g0, p0 = pend.pop(0)
                        emit_O(h0_, qb, g0, p0, oTs[h0_])
            if dq:
                dq.pop(0)()
            elif fq:
                fq.pop(0)()
            for h0_, g0, p0 in pend:
                emit_O(h0_, qb, g0, p0, oTs[h0_])
            for h in hp:
                emit_norm(h, qb, oTs[h])
            if qb == NQB - 1 and hp == (0, 1):
                # last q-block: its kc0 out-proj half only needs heads 0,1 —
                # feed it to the tail-end groups as free filler
                fq.extend(stage6_units(NQB - 1, [fillp], kcs=(0,)))
        while dq:
            dq.pop(0)()
    while fq:
        fq.pop(0)()
    # tail: kc1 half of the last q-block -> yT2 (host-summed). A few junk
    # matmuls bridge the PE-idle norm window (keeps the HAM clock warm), then
    # the 8 units pipeline over two psum pools with alternating copy engines
    # and alternating DMA queues
    emit_warm(6, pool=widep)
    for un in stage6_units(NQB - 1, [fillp, widep], kcs=(1,), out_t=yT2,
                           alt_copy=True, alt_dma=True):
        un()


def build_nc():
    from contextlib import ExitStack

    nc = bacc.Bacc("TRN2", target_bir_lowering=False)
    xT = nc.dram_tensor("xT", [C, T], BF, kind="ExternalInput")
    wqk = nc.dram_tensor("wqk", [C, 2 * CS], BF, kind="ExternalInput")
    wv = nc.dram_tensor("wv", [C, CS], BF, kind="ExternalInput")
    bqk = nc.dram_tensor("bqk", [2 * CS, 1], F32, kind="ExternalInput")
    wp = nc.dram_tensor("wp", [CS, C], BF, kind="ExternalInput")
    masks = nc.dram_tensor("masks", [128, 128], BF, kind="ExternalInput")
    yT = nc.dram_tensor("yT", [C, T], F16, kind="ExternalOutput")
    yT2 = nc.dram_tensor("yT2", [C, T], F16, kind="ExternalOutput")
    with tile.TileContext(nc) as tc:
        with nc.allow_low_precision(reason="bf16 matmul inputs; accumulation stays fp32 in PSUM"):
            with ExitStack() as ctx:
                _build_body(nc, tc, ctx, xT, wqk, wv, bqk, wp, masks, yT, yT2)
    nc.compile()
    return nc


def make_mask01():
    r = np.arange(128)[:, None]
    c = np.arange(128)[None, :]
    return np.where(r <= c, 1.0, 0.0).astype(BF_NP)


def make_in_maps(x, W_qkv, b_qkv, W_proj):
    scale = np.float32(1.0 / np.sqrt(D))
    mask_h = make_mask01()
    in_maps = []
    for i in range(NCORES):
        b, g = divmod(i, HPC)
        cs0 = g * CS
        wq = W_qkv[:, cs0:cs0 + CS] * scale
        wk = W_qkv[:, C + cs0:C + cs0 + CS]
        bq = b_qkv[cs0:cs0 + CS] * scale
        bk = b_qkv[C + cs0:C + cs0 + CS]
        in_maps.append({
            "xT": np.ascontiguousarray(x[b].T).astype(BF_NP),
            "wqk": np.concatenate([wq, wk], axis=1).astype(BF_NP),
            "wv": np.ascontiguousarray(W_qkv[:, 2 * C + cs0:2 * C + cs0 + CS]).astype(BF_NP),
            "bqk": np.concatenate([bq, bk])[:, None].astype(np.float32),
            "wp": np.ascontiguousarray(W_proj[cs0:cs0 + CS, :]).astype(BF_NP),
            "masks": mask_h,
        })
    return in_maps


_NC_CACHE = None


def _get_nc():
    global _NC_CACHE
    if _NC_CACHE is None:
        _NC_CACHE = build_nc()
    return _NC_CACHE


def gather(results, b_qkv, W_proj, b_proj):
    Y = np.zeros((B, T, C), np.float32)
    for i in range(NCORES):
        Y[i // HPC] += results[i]["yT"].T.astype(np.float32)
        qb3 = (NQB - 1) * QB
        Y[i // HPC][qb3:] += results[i]["yT2"].T[qb3:].astype(np.float32)
    Y += (b_qkv[2 * C:].astype(np.float32) @ W_proj.astype(np.float32)
          + b_proj.astype(np.float32))[None, None, :]
    return Y


def kernel(x, W_qkv, b_qkv, W_proj, b_proj):
    global LAST_RESULT
    x = np.asarray(x, np.float32)
    W_qkv = np.asarray(W_qkv, np.float32)
    b_qkv = np.asarray(b_qkv, np.float32)
    W_proj = np.asarray(W_proj, np.float32)
    b_proj = np.asarray(b_proj, np.float32)

    nc = _get_nc()
    in_maps = make_in_maps(x, W_qkv, b_qkv, W_proj)
    res = run_bass_kernel_spmd(nc, in_maps, list(range(NCORES)), trace=TRACE)
    LAST_RESULT = res
    if TRACE and res.exec_time_ns is not None:
        print(f"HW exec time: {res.exec_time_ns} ns")
    return gather(res.results, b_qkv, W_proj, b_proj)


# revision 26
# speedup vs baseline: 1.1423x; 1.0921x over previous
"""Causal multi-head attention (B=2, T=2048, C=1024, H=16, d=64) on 8 trn2 cores.

Sharding: core i -> (batch b = i//4, head group g = i%4, 4 heads/core).
Data parallel over B, tensor parallel over heads; the out-proj partial sums
(contraction over this core's 256 channels) are reduced on the host during
the gather step, along with b_proj and the analytically-folded V bias.

All matmul operands are bfloat16 (full PE rate at any N; fp32 PSUM accum);
the y partials are DMA'd from PSUM in fp32 and summed on the host.

Device kernel works in [feature, token] (transposed) layout; ascending
q-blocks with stage-1 interleaved in chunks so the PE ramps with the DMA:
  stage 0: a few junk warmup matmuls on an uninitialized tile keep the PE
           HAM activity monitor busy during the initial DMA wait, so real
           matmuls start at the warm 2.4 GHz clock instead of 1.2 GHz
  stage 1 (chunk tc4): Q^T,K^T cols = (Wqk)^T x^T (+bias, 1/sqrt(d) folded
           into Wq host-side); V t-tiles 4*tc4..4*tc4+3 in natural layout.
           Chunk 0 upfront; chunk qb+1 emitted inside q-block qb's head loop.
  stage 2: S^T[j,q] = K_h^T.T @ Q_h^T  per head; the head pair's matmuls
           (K=64, row groups 0/64) are emitted back-to-back alternating row
           groups so the PE runs them CONCURRENTLY (2x row tiling)
  stage 3: P^T = exp(S^T); the causal mask is applied AFTER exp as a 0/1
           multiply on the 128-wide diagonal block (off the S->P critical
           path; hidden behind the O-matmul lag)
  stage 4: outT[128,q] = [V_h | 1s(64)]^T.T @ P^T accumulated over j tiles
           (rows 64:128 = softmax denominator Z replicated 64x); O-matmuls
           are emitted a few groups behind their exp so the in-order PE
           never waits on ACT
  stage 5: att^T = outT[0:64] * reciprocal(outT[64:128])  (pure DVE)
  stage 6: y^T = Wp.T @ att^T -> fp16 out; host sums.
"""

import numpy as np
import ml_dtypes

import concourse.bass as bass
import concourse.mybir as mybir
from concourse import bacc
import concourse.tile as tile
from concourse.bass_utils import run_bass_kernel_spmd

B, T, C, H, D = 2, 2048, 1024, 16, 64
NCORES = 8
HPC = 4            # heads per core
CS = HPC * D       # 256 channels per core (per Q/K/V block)
KT = C // 128      # 8 contraction tiles for the projections
NT = T // 128      # 16 token tiles of 128
QB = 512           # query block (psum bank width in fp32)
NQB = T // QB      # 4 query blocks

F32 = mybir.dt.float32
F16 = mybir.dt.float16
BF = mybir.dt.bfloat16
BF_NP = ml_dtypes.bfloat16

TRACE = False
LAST_RESULT = None


def _build_body(nc, tc, ctx, xT, wqk, wv, bqk, wp, masks, yT, yT2):
    AF = mybir.ActivationFunctionType

    persist = ctx.enter_context(tc.tile_pool(name="persist", bufs=1))

    wqk_sb = persist.tile([128, KT, 2 * CS], BF, tag="wqk", name="wqk")
    wv_sb = persist.tile([128, KT, CS], BF, tag="wv", name="wv")
    bqk_sb = persist.tile([128, 4], F32, tag="bqk", name="bqk")
    wp_sb = persist.tile([128, 2, C], BF, tag="wp", name="wp")
    mask_sb = persist.tile([128, 128], BF, tag="mask", name="mask_sb")
    warm_sb = persist.tile([128, QB], BF, tag="warm", name="warm_sb")
    qT_sb = [persist.tile([128, T], BF, tag=f"qT{i}", name=f"qT{i}") for i in range(2)]
    # per-head K tiles, zero-padded in the other head's 64 partitions: the
    # S-matmul then runs with K=128 (full 128x128 tiling mode, FWL-eligible)
    # instead of K=64, eliminating the ~90ns 64-row-mode switch the PE pays
    # entering/leaving each S matmul; the zero rows contribute nothing
    kTp_sb = [persist.tile([128, T], BF, tag=f"kTp{h}", name=f"kTp{h}")
              for h in range(HPC)]
    # per j-tile: 4 heads x (64 ones cols | 64 V cols); the ones columns
    # replicate the softmax denominator into outT rows 0:64 (base partition 0
    # so reciprocal_approx_fast reads it correctly; custom DVE ops ignore a
    # nonzero input base partition)
    v_sb = [persist.tile([128, HPC, 2 * D], BF, tag=f"v{t}", name=f"v{t}") for t in range(NT)]
    attT_sb = [persist.tile([128, T], BF, tag=f"attT{i}", name=f"attT{i}") for i in range(2)]
    xT_sb = persist.tile([128, KT, T], BF, tag="xT", name="xT")

    widep = ctx.enter_context(tc.tile_pool(name="wide", bufs=2, space="PSUM"))
    oTp = ctx.enter_context(tc.tile_pool(name="oT", bufs=2, space="PSUM"))
    fillp = ctx.enter_context(tc.tile_pool(name="fill", bufs=1, space="PSUM"))
    pTp = ctx.enter_context(tc.tile_pool(name="pT", bufs=8))
    smallp = ctx.enter_context(tc.tile_pool(name="small", bufs=8))
    ysp = ctx.enter_context(tc.tile_pool(name="ystage", bufs=6))

    # ---------------- PE warmup ----------------
    # junk matmuls (no data deps beyond one tiny memset) issue right after the
    # framework preamble and keep the PE busy through the DMA ramp, so the HAM
    # un-throttles to 2.4 GHz before the first real matmul
    nc.gpsimd.memset(warm_sb[:, :], 0.0)

    def emit_warm(n, pool=fillp):
        wps = pool.tile([128, 2, QB], F32, tag="sT", name="warmps")
        for _ in range(n):
            nc.tensor.matmul(wps[:, 0, :], lhsT=warm_sb[:, 0:128],
                             rhs=warm_sb[:, :], start=True, stop=True)

    # ---------------- DMAs ----------------
    # each dma_start costs ~600ns of serial issue time on its engine, so the
    # per-k tiles are merged into wide transfers, issues are split across the
    # two HWDGE engines (SP + ACT), and the big x column-tail transfers are
    # issued from the gpsimd SWDGE queue BEFORE its memsets so the fabric
    # starts streaming them early
    xTr = xT.rearrange("(k p) t -> p k t", k=KT)
    wqkr = wqk.rearrange("(k p) c -> p k c", k=KT)
    nc.sync.dma_start(out=xT_sb[:, 0:1, 0:QB], in_=xTr[:, 0:1, 0:QB])
    nc.scalar.dma_start(out=wqk_sb[:, 0:1, :], in_=wqkr[:, 0:1, :])
    nc.sync.dma_start(out=xT_sb[:, 1:2, 0:QB], in_=xTr[:, 1:2, 0:QB])
    nc.scalar.dma_start(out=wqk_sb[:, 1:2, :], in_=wqkr[:, 1:2, :])
    nc.sync.dma_start(out=xT_sb[:, 2:4, 0:QB], in_=xTr[:, 2:4, 0:QB])
    nc.scalar.dma_start(out=wqk_sb[:, 2:4, :], in_=wqkr[:, 2:4, :])
    nc.sync.dma_start(out=xT_sb[:, 4:KT, 0:QB], in_=xTr[:, 4:KT, 0:QB])
    nc.scalar.dma_start(out=wqk_sb[:, 4:KT, :], in_=wqkr[:, 4:KT, :])
    nc.scalar.dma_start(out=mask_sb[:, :], in_=masks[:, :])
    nc.scalar.dma_start(out=bqk_sb[:, :], in_=bqk.rearrange("(c p) one -> p (c one)", c=4))
    nc.scalar.dma_start(out=wv_sb[:, :, :], in_=wv.rearrange("(k p) c -> p k c", k=KT))
    for t in range(NT):
        nc.gpsimd.memset(v_sb[t][:, :, :], 1.0)
    nc.gpsimd.dma_start(out=wp_sb[:, :, :], in_=wp.rearrange("(k p) c -> p k c", k=2))
    # x column tail (cols 512:2048) as per-k 384KB pieces with 3KB lines:
    # each chunk-unit kstep waits only its own k piece, and pieces arrive
    # k-ascending in consumption order
    for k in range(KT):
        nc.gpsimd.dma_start(
            out=xT_sb[:, k:k + 1, QB:T],
            in_=xTr[:, k:k + 1, QB:T],
        )
    # zero the pad halves of the per-head K tiles on the (initially idle)
    # vector engine; done long before the first S matmul reads them
    for h in range(HPC):
        pad = slice(D, 128) if h % 2 == 0 else slice(0, D)
        nc.vector.memset(kTp_sb[h][pad, :], 0.0)

    # ---------------- filler work units ----------------
    # stage-1 chunks and stage-6 out-proj are chopped into ~0.2-0.4us PE
    # units and drained between attention groups, filling every ACT-lag
    # bubble while keeping the in-order PE busy
    def qk_pair_units(tc4, pair, pool):
        ref = {}
        units = []

        def bias_add(m):
            ps = ref["ps"]
            ct = pair[m]
            cols = slice(tc4 * QB, (tc4 + 1) * QB)
            if ct < 2:
                nc.vector.tensor_scalar_add(
                    qT_sb[ct][:, cols], ps[:, m, :], bqk_sb[:, ct:ct + 1]
                )
            else:
                # K channels split per head into the zero-padded tiles
                # (same lanes: even head rows 0:64, odd head rows 64:128)
                h0, h1 = 2 * (ct - 2), 2 * (ct - 2) + 1
                nc.vector.tensor_scalar_add(
                    kTp_sb[h0][0:D, cols], ps[0:D, m, :], bqk_sb[0:D, ct:ct + 1]
                )
                nc.vector.tensor_scalar_add(
                    kTp_sb[h1][D:128, cols], ps[D:128, m, :],
                    bqk_sb[D:128, ct:ct + 1]
                )

        def kstep(k):
            def u():
                if k == 0:
                    ref["ps"] = pool.tile([128, 2, QB], F32, tag="sT", name="wps")
                ps = ref["ps"]
                for m, ct in enumerate(pair):
                    nc.tensor.matmul(
                        ps[:, m, :],
                        lhsT=wqk_sb[:, k, ct * 128:(ct + 1) * 128],
                        rhs=xT_sb[:, k, tc4 * QB:(tc4 + 1) * QB],
                        start=(k == 0),
                        stop=(k == KT - 1),
                    )
                if k == KT - 1:
                    bias_add(0)   # m0 drain starts a unit early
            return u

        units = [kstep(k) for k in range(KT)]
        units.append(lambda: bias_add(1))
        return units

    def v_pair_units(tc4, tp, pool, tag="sT"):
        t0 = 4 * tc4 + 2 * tp
        ref = {}

        def copies(m):
            # one contiguous copy per head: strided/rearranged dst APs
            # mis-register their write deps (O-matmuls then race the copy)
            ps = ref["ps"]
            t = t0 + m
            for h in range(HPC):
                nc.vector.tensor_copy(
                    v_sb[t][:, h, D:2 * D], ps[:, m, h * D:(h + 1) * D]
                )

        def kstep(k):
            def u():
                if k == 0:
                    ref["ps"] = pool.tile([128, 2, QB], F32, tag=tag, name="wps")
                ps = ref["ps"]
                for m in range(2):
                    t = t0 + m
                    nc.tensor.matmul(
                        ps[:, m, 0:CS],
                        lhsT=xT_sb[:, k, t * 128:(t + 1) * 128],
                        rhs=wv_sb[:, k, :],
                        start=(k == 0),
                        stop=(k == KT - 1),
                    )
                if k == KT - 1:
                    copies(0)   # t0 drain starts a unit early
            return u

        units = [kstep(k) for k in range(KT)]
        units.append(lambda: copies(1))
        return units

    def chunk_units(tc4, pool):
        units = []
        for pair in ((0, 2), (1, 3)):
            units += qk_pair_units(tc4, pair, pool)
        for tp in range(2):
            units += v_pair_units(tc4, tp, pool)
        return units

    def stage6_units(sqb, pools, kcs=(0, 1), out_t=None, alt_copy=False,
                     alt_dma=False):
        # one combined unit per et: the CAST is emitted right after its chain
        # so it lands early in the in-order vector queue (no convoy behind
        # norm-muls); consecutive units are spaced >=2 groups by the fq drain
        ot = out_t if out_t is not None else yT
        units = []
        for et in range(C // 128):
            def u(et=et):
                pool = pools[et % len(pools)]
                yps_w = pool.tile([128, 2, QB], F32, tag="sT", name="yps")
                yps_t = yps_w[:, 0, :]
                for j, kc in enumerate(kcs):
                    nc.tensor.matmul(
                        yps_t,
                        lhsT=wp_sb[:, kc, et * 128:(et + 1) * 128],
                        rhs=attT_sb[kc][:, sqb * QB:(sqb + 1) * QB],
                        start=(j == 0),
                        stop=(j == len(kcs) - 1),
                    )
                ys = ysp.tile([128, QB], F16, tag="ys", name="ys")
                if alt_copy and et % 2 == 1:
                    nc.scalar.activation(ys[:, :], yps_t, AF.Copy)
                else:
                    nc.vector.tensor_copy(ys[:, :], yps_t)
                eng = nc.scalar if (alt_dma and et % 2 == 1) else nc.sync
                eng.dma_start(
                    out=ot[et * 128:(et + 1) * 128, sqb * QB:(sqb + 1) * QB],
                    in_=ys[:, :],
                )
            units.append(u)
        return units

    # ---------------- attention piece emitters ----------------
    def emit_S(h, qb, grp, sT):
        ktile = kTp_sb[h]
        qtile = qT_sb[h // 2]
        diag = grp >= 2 * (qb + 1) - 2
        for m in range(2):
            jt = grp * 2 + m
            c0 = 128 * (jt - 4 * qb) if diag else 0
            nc.tensor.matmul(
                sT[:, m, c0:QB],
                lhsT=ktile[:, jt * 128:(jt + 1) * 128],
                rhs=qtile[:, qb * QB + c0:(qb + 1) * QB],
                start=True,
                stop=True,
            )

    def emit_exp(h, qb, grp, sT, pT):
        diag = grp >= 2 * (qb + 1) - 2
        if diag:
            # ONE fused exp for both m j-tiles starting at m0's causal
            # offset; the m1 columns below its own offset become junk in pT
            # but the O-matmul only streams pT from its per-tile offset, so
            # they are never read. Saves the ~293ns fixed ACTIVATE overhead
            # of a second instruction on the pacing ACT engine.
            c0 = 128 * (grp * 2 - 4 * qb)
            nc.scalar.activation(pT[:, :, c0:QB], sT[:, :, c0:QB], AF.Exp)
            # 0/1 causal mask on the diagonal 128-block, after exp so the
            # S->P critical path has no DVE hop; hidden by the O lag
            for m in range(2):
                cm = 128 * (grp * 2 + m - 4 * qb)
                nc.vector.tensor_mul(
                    pT[:, m, cm:cm + 128], pT[:, m, cm:cm + 128], mask_sb[:, :]
                )
        else:
            nc.scalar.activation(pT[:, :, :], sT[:, :, :], AF.Exp)

    def emit_O(h, qb, grp, pT, oT):
        njt = 4 * (qb + 1)
        diag = grp >= 2 * (qb + 1) - 2
        for m in range(2):
            jt = grp * 2 + m
            c0 = 128 * (jt - 4 * qb) if diag else 0
            nc.tensor.matmul(
                oT[:, c0:QB],
                lhsT=v_sb[jt][:, h, :],
                rhs=pT[:, m, c0:QB],
                start=(jt == 0),
                stop=(jt == njt - 1),
            )

    def emit_norm(h, qb, oT):
        po = (h % 2) * D
        rz = smallp.tile([D, QB], F32, tag="rz", name="rz")
        nc.vector.reciprocal_approx_fast(out=rz[:, :], in_=oT[0:D, :])
        nc.vector.tensor_mul(
            attT_sb[h // 2][po:po + D, qb * QB:(qb + 1) * QB],
            oT[D:2 * D, :],
            rz[:, :],
        )

    # ---------------- main schedule ----------------
    # chunk 0 upfront (the DMA ramp); spread its 4 psum tiles over both pools
    for un in qk_pair_units(0, (0, 2), widep):
        un()
    for un in qk_pair_units(0, (1, 3), widep):
        un()
    for un in v_pair_units(0, 0, fillp):
        un()
    for un in v_pair_units(0, 1, fillp):
        un()

    dq = []   # deadline units: chunk(qb+1) must finish within qb
    fq = []   # free units: stage6, drained evenly over ALL remaining groups

    total_groups = HPC * sum(2 * (qb + 1) for qb in range(NQB))
    state = {"tg_left": total_groups}

    def drain(qb_groups_left):
        n = -(-len(dq) // qb_groups_left) if qb_groups_left > 0 else len(dq)
        for _ in range(min(n, len(dq))):
            dq.pop(0)()
        nf = -(-len(fq) // state["tg_left"]) if state["tg_left"] > 0 else len(fq)
        for _ in range(min(nf, len(fq))):
            fq.pop(0)()

    for qb in range(NQB):
        ngr = 2 * (qb + 1)
        if qb >= 1:
            fq.extend(stage6_units(qb - 1, [fillp]))
        if qb < NQB - 1:
            dq.extend(chunk_units(qb + 1, fillp))
        g_left = HPC * ngr
        # heads run in pairs, groups round-robin across the pair: exp(h,g)
        # gets two heads' worth of PE time before its O-matmuls are due, so
        # the single ACT stream never gates the in-order PE
        for hp in ((0, 1), (2, 3)):
            oTs = {h: oTp.tile([128, QB], F32, tag="oT", name="oT") for h in hp}
            pend = []
            for grp in range(ngr):
                for h in hp:
                    sT = widep.tile([128, 2, QB], F32, tag="sT", name="sT")
                    emit_S(h, qb, grp, sT)
                    pT = pTp.tile([128, 2, QB], BF, tag="pT", name="pT")
                    emit_exp(h, qb, grp, sT, pT)
                    drain(g_left)
                    g_left -= 1
                    state["tg_left"] -= 1
                    pend.append((h, grp, pT))
                    if len(pend) > 4:
                        h0_, g0, p0 = pend.pop(0)
                        emit_O(h0_, qb, g0, p0, oTs[h0_])
            if dq:
                dq.pop(0)()
            elif fq:
                fq.pop(0)()
            for h0_, g0, p0 in pend:
                emit_O(h0_, qb, g0, p0, oTs[h0_])
            for h in hp:
                emit_norm(h, qb, oTs[h])
            if qb == NQB - 1 and hp == (0, 1):
                # last q-block: its kc0 out-proj half only needs heads 0,1 —
                # feed it to the tail-end groups as free filler
                fq.extend(stage6_units(NQB - 1, [fillp], kcs=(0,)))
        while dq:
            dq.pop(0)()
    while fq:
        fq.pop(0)()
    # tail: kc1 half of the last q-block -> yT2 (host-summed). A few junk
    # matmuls bridge the PE-idle norm window (keeps the HAM clock warm), then
    # the 8 units pipeline over two psum pools with alternating copy engines
    # and alternating DMA queues
    emit_warm(6, pool=widep)
    for un in stage6_units(NQB - 1, [fillp, widep], kcs=(1,), out_t=yT2,
                           alt_copy=True, alt_dma=True):
        un()


def build_nc():
    from contextlib import ExitStack

    nc = bacc.Bacc("TRN2", target_bir_lowering=False)
    xT = nc.dram_tensor("xT", [C, T], BF, kind="ExternalInput")
    wqk = nc.dram_tensor("wqk", [C, 2 * CS], BF, kind="ExternalInput")
    wv = nc.dram_tensor("wv", [C, CS], BF, kind="ExternalInput")
    bqk = nc.dram_tensor("bqk", [2 * CS, 1], F32, kind="ExternalInput")
    wp = nc.dram_tensor("wp", [CS, C], BF, kind="ExternalInput")
    masks = nc.dram_tensor("masks", [128, 128], BF, kind="ExternalInput")
    yT = nc.dram_tensor("yT", [C, T], F16, kind="ExternalOutput")
    yT2 = nc.dram_tensor("yT2", [C, T], F16, kind="ExternalOutput")
    with tile.TileContext(nc) as tc:
        with nc.allow_low_precision(reason="bf16 matmul inputs; accumulation stays fp32 in PSUM"):
            with ExitStack() as ctx:
                _build_body(nc, tc, ctx, xT, wqk, wv, bqk, wp, masks, yT, yT2)
    nc.compile()
    return nc


def make_mask01():
    r = np.arange(128)[:, None]
    c = np.arange(128)[None, :]
    return np.where(r <= c, 1.0, 0.0).astype(BF_NP)


def make_in_maps(x, W_qkv, b_qkv, W_proj):
    scale = np.float32(1.0 / np.sqrt(D))
    mask_h = make_mask01()
    in_maps = []
    for i in range(NCORES):
        b, g = divmod(i, HPC)
        cs0 = g * CS
        wq = W_qkv[:, cs0:cs0 + CS] * scale
        wk = W_qkv[:, C + cs0:C + cs0 + CS]
        bq = b_qkv[cs0:cs0 + CS] * scale
        bk = b_qkv[C + cs0:C + cs0 + CS]
        in_maps.append({
            "xT": np.ascontiguousarray(x[b].T).astype(BF_NP),
            "wqk": np.concatenate([wq, wk], axis=1).astype(BF_NP),
            "wv": np.ascontiguousarray(W_qkv[:, 2 * C + cs0:2 * C + cs0 + CS]).astype(BF_NP),
            "bqk": np.concatenate([bq, bk])[:, None].astype(np.float32),
            "wp": np.ascontiguousarray(W_proj[cs0:cs0 + CS, :]).astype(BF_NP),
            "masks": mask_h,
        })
    return in_maps


_NC_CACHE = None


def _get_nc():
    global _NC_CACHE
    if _NC_CACHE is None:
        _NC_CACHE = build_nc()
    return _NC_CACHE


def gather(results, b_qkv, W_proj, b_proj):
    Y = np.zeros((B, T, C), np.float32)
    for i in range(NCORES):
        Y[i // HPC] += results[i]["yT"].T.astype(np.float32)
        qb3 = (NQB - 1) * QB
        Y[i // HPC][qb3:] += results[i]["yT2"].T[qb3:].astype(np.float32)
    Y += (b_qkv[2 * C:].astype(np.float32) @ W_proj.astype(np.float32)
          + b_proj.astype(np.float32))[None, None, :]
    return Y


def kernel(x, W_qkv, b_qkv, W_proj, b_proj):
    global LAST_RESULT
    x = np.asarray(x, np.float32)
    W_qkv = np.asarray(W_qkv, np.float32)
    b_qkv = np.asarray(b_qkv, np.float32)
    W_proj = np.asarray(W_proj, np.float32)
    b_proj = np.asarray(b_proj, np.float32)

    nc = _get_nc()
    in_maps = make_in_maps(x, W_qkv, b_qkv, W_proj)
    res = run_bass_kernel_spmd(nc, in_maps, list(range(NCORES)), trace=TRACE)
    LAST_RESULT = res
    if TRACE and res.exec_time_ns is not None:
        print(f"HW exec time: {res.exec_time_ns} ns")
    return gather(res.results, b_qkv, W_proj, b_proj)


# revision 30
# speedup vs baseline: 1.2021x; 1.0523x over previous
"""Causal multi-head attention (B=2, T=2048, C=1024, H=16, d=64) on 8 trn2 cores.

Sharding: core i -> (batch b = i//4, head group g = i%4, 4 heads/core).
Data parallel over B, tensor parallel over heads; the out-proj partial sums
(contraction over this core's 256 channels) are reduced on the host during
the gather step, along with b_proj and the analytically-folded V bias.

All matmul operands are bfloat16 (full PE rate at any N; fp32 PSUM accum);
the y partials are DMA'd from PSUM in fp32 and summed on the host.

Device kernel works in [feature, token] (transposed) layout; ascending
q-blocks with stage-1 interleaved in chunks so the PE ramps with the DMA:
  stage 0: a few junk warmup matmuls on an uninitialized tile keep the PE
           HAM activity monitor busy during the initial DMA wait, so real
           matmuls start at the warm 2.4 GHz clock instead of 1.2 GHz
  stage 1 (chunk tc4): Q^T,K^T cols = (Wqk)^T x^T (+bias, 1/sqrt(d) folded
           into Wq host-side); V t-tiles 4*tc4..4*tc4+3 in natural layout.
           Chunk 0 upfront; chunk qb+1 emitted inside q-block qb's head loop.
  stage 2: S^T[j,q] = K_h^T.T @ Q_h^T  per head; the head pair's matmuls
           (K=64, row groups 0/64) are emitted back-to-back alternating row
           groups so the PE runs them CONCURRENTLY (2x row tiling)
  stage 3: P^T = exp(S^T); the causal mask is applied AFTER exp as a 0/1
           multiply on the 128-wide diagonal block (off the S->P critical
           path; hidden behind the O-matmul lag)
  stage 4: outT[128,q] = [V_h | 1s(64)]^T.T @ P^T accumulated over j tiles
           (rows 64:128 = softmax denominator Z replicated 64x); O-matmuls
           are emitted a few groups behind their exp so the in-order PE
           never waits on ACT
  stage 5: att^T = outT[0:64] * reciprocal(outT[64:128])  (pure DVE)
  stage 6: y^T = Wp.T @ att^T -> fp16 out; host sums.
"""

import numpy as np
import ml_dtypes

import concourse.bass as bass
import concourse.mybir as mybir
from concourse import bacc
import concourse.tile as tile
from concourse.bass_utils import run_bass_kernel_spmd

B, T, C, H, D = 2, 2048, 1024, 16, 64
NCORES = 8
HPC = 4            # heads per core
CS = HPC * D       # 256 channels per core (per Q/K/V block)
KT = C // 128      # 8 contraction tiles for the projections
NT = T // 128      # 16 token tiles of 128
QB = 512           # query block (psum bank width in fp32)
NQB = T // QB      # 4 query blocks

F32 = mybir.dt.float32
F16 = mybir.dt.float16
BF = mybir.dt.bfloat16
BF_NP = ml_dtypes.bfloat16

TRACE = False
LAST_RESULT = None


def _build_body(nc, tc, ctx, xT, wqk, wv, bqk, wp, masks, yT, yT2):
    AF = mybir.ActivationFunctionType

    persist = ctx.enter_context(tc.tile_pool(name="persist", bufs=1))

    wqk_sb = persist.tile([128, KT, 2 * CS], BF, tag="wqk", name="wqk")
    wv_sb = persist.tile([128, KT, CS], BF, tag="wv", name="wv")
    bqk_sb = persist.tile([128, 4], F32, tag="bqk", name="bqk")
    wp_sb = persist.tile([128, 2, C], BF, tag="wp", name="wp")
    mask_sb = persist.tile([128, 128], BF, tag="mask", name="mask_sb")
    warm_sb = persist.tile([128, QB], BF, tag="warm", name="warm_sb")
    qT_sb = [persist.tile([128, T], BF, tag=f"qT{i}", name=f"qT{i}") for i in range(2)]
    kT_sb = [persist.tile([128, T], BF, tag=f"kT{i}", name=f"kT{i}") for i in range(2)]
    # per j-tile: 4 heads x (64 ones cols | 64 V cols); the ones columns
    # replicate the softmax denominator into outT rows 0:64 (base partition 0
    # so reciprocal_approx_fast reads it correctly; custom DVE ops ignore a
    # nonzero input base partition)
    v_sb = [persist.tile([128, HPC, 2 * D], BF, tag=f"v{t}", name=f"v{t}") for t in range(NT)]
    attT_sb = [persist.tile([128, T], BF, tag=f"attT{i}", name=f"attT{i}") for i in range(2)]
    xT_sb = persist.tile([128, KT, T], BF, tag="xT", name="xT")

    widep = ctx.enter_context(tc.tile_pool(name="wide", bufs=2, space="PSUM"))
    oTp = ctx.enter_context(tc.tile_pool(name="oT", bufs=2, space="PSUM"))
    fillp = ctx.enter_context(tc.tile_pool(name="fill", bufs=1, space="PSUM"))
    pTp = ctx.enter_context(tc.tile_pool(name="pT", bufs=8))
    smallp = ctx.enter_context(tc.tile_pool(name="small", bufs=8))
    ysp = ctx.enter_context(tc.tile_pool(name="ystage", bufs=6))

    # ---------------- PE warmup ----------------
    # junk matmuls (no data deps beyond one tiny memset) issue right after the
    # framework preamble and keep the PE busy through the DMA ramp, so the HAM
    # un-throttles to 2.4 GHz before the first real matmul
    nc.gpsimd.memset(warm_sb[:, :], 0.0)

    def emit_warm(n, pool=fillp):
        wps = pool.tile([128, 2, QB], F32, tag="sT", name="warmps")
        for _ in range(n):
            nc.tensor.matmul(wps[:, 0, :], lhsT=warm_sb[:, 0:128],
                             rhs=warm_sb[:, :], start=True, stop=True)

    # ---------------- DMAs ----------------
    # each dma_start costs ~600ns of serial issue time on its engine, so the
    # per-k tiles are merged into wide transfers, issues are split across the
    # two HWDGE engines (SP + ACT), and the big x column-tail transfers are
    # issued from the gpsimd SWDGE queue BEFORE its memsets so the fabric
    # starts streaming them early
    xTr = xT.rearrange("(k p) t -> p k t", k=KT)
    wqkr = wqk.rearrange("(k p) c -> p k c", k=KT)
    nc.sync.dma_start(out=xT_sb[:, 0:1, 0:QB], in_=xTr[:, 0:1, 0:QB])
    nc.scalar.dma_start(out=wqk_sb[:, 0:1, :], in_=wqkr[:, 0:1, :])
    nc.sync.dma_start(out=xT_sb[:, 1:2, 0:QB], in_=xTr[:, 1:2, 0:QB])
    nc.scalar.dma_start(out=wqk_sb[:, 1:2, :], in_=wqkr[:, 1:2, :])
    nc.sync.dma_start(out=xT_sb[:, 2:4, 0:QB], in_=xTr[:, 2:4, 0:QB])
    nc.scalar.dma_start(out=wqk_sb[:, 2:4, :], in_=wqkr[:, 2:4, :])
    nc.sync.dma_start(out=xT_sb[:, 4:KT, 0:QB], in_=xTr[:, 4:KT, 0:QB])
    nc.scalar.dma_start(out=wqk_sb[:, 4:KT, :], in_=wqkr[:, 4:KT, :])
    nc.scalar.dma_start(out=mask_sb[:, :], in_=masks[:, :])
    nc.scalar.dma_start(out=bqk_sb[:, :], in_=bqk.rearrange("(c p) one -> p (c one)", c=4))
    nc.scalar.dma_start(out=wv_sb[:, :, :], in_=wv.rearrange("(k p) c -> p k c", k=KT))
    for t in range(NT):
        nc.gpsimd.memset(v_sb[t][:, :, :], 1.0)
    nc.gpsimd.dma_start(out=wp_sb[:, :, :], in_=wp.rearrange("(k p) c -> p k c", k=2))
    # x column tail (cols 512:2048) as per-k 384KB pieces with 3KB lines:
    # each chunk-unit kstep waits only its own k piece, and pieces arrive
    # k-ascending in consumption order
    for k in range(KT):
        nc.gpsimd.dma_start(
            out=xT_sb[:, k:k + 1, QB:T],
            in_=xTr[:, k:k + 1, QB:T],
        )


    # ---------------- filler work units ----------------
    # stage-1 chunks and stage-6 out-proj are chopped into ~0.2-0.4us PE
    # units and drained between attention groups, filling every ACT-lag
    # bubble while keeping the in-order PE busy
    def qk_pair_units(tc4, pair, pool):
        ref = {}
        units = []

        def bias_add(m):
            ps = ref["ps"]
            ct = pair[m]
            dst = qT_sb[ct] if ct < 2 else kT_sb[ct - 2]
            nc.vector.tensor_scalar_add(
                dst[:, tc4 * QB:(tc4 + 1) * QB], ps[:, m, :], bqk_sb[:, ct:ct + 1]
            )

        def kstep(k):
            def u():
                if k == 0:
                    ref["ps"] = pool.tile([128, 2, QB], F32, tag="sT", name="wps")
                ps = ref["ps"]
                for m, ct in enumerate(pair):
                    nc.tensor.matmul(
                        ps[:, m, :],
                        lhsT=wqk_sb[:, k, ct * 128:(ct + 1) * 128],
                        rhs=xT_sb[:, k, tc4 * QB:(tc4 + 1) * QB],
                        start=(k == 0),
                        stop=(k == KT - 1),
                    )
                if k == KT - 1:
                    bias_add(0)   # m0 drain starts a unit early
            return u

        units = [kstep(k) for k in range(KT)]
        units.append(lambda: bias_add(1))
        return units

    def v_pair_units(tc4, tp, pool, tag="sT"):
        t0 = 4 * tc4 + 2 * tp
        ref = {}

        def copies(m):
            # one contiguous copy per head: strided/rearranged dst APs
            # mis-register their write deps (O-matmuls then race the copy)
            ps = ref["ps"]
            t = t0 + m
            for h in range(HPC):
                nc.vector.tensor_copy(
                    v_sb[t][:, h, D:2 * D], ps[:, m, h * D:(h + 1) * D]
                )

        def kstep(k):
            def u():
                if k == 0:
                    ref["ps"] = pool.tile([128, 2, QB], F32, tag=tag, name="wps")
                ps = ref["ps"]
                for m in range(2):
                    t = t0 + m
                    nc.tensor.matmul(
                        ps[:, m, 0:CS],
                        lhsT=xT_sb[:, k, t * 128:(t + 1) * 128],
                        rhs=wv_sb[:, k, :],
                        start=(k == 0),
                        stop=(k == KT - 1),
                    )
                if k == KT - 1:
                    copies(0)   # t0 drain starts a unit early
            return u

        units = [kstep(k) for k in range(KT)]
        units.append(lambda: copies(1))
        return units

    def chunk_units(tc4, pool):
        units = []
        for pair in ((0, 2), (1, 3)):
            units += qk_pair_units(tc4, pair, pool)
        for tp in range(2):
            units += v_pair_units(tc4, tp, pool)
        return units

    def stage6_units(sqb, pools, kcs=(0, 1), out_t=None, alt_copy=False,
                     alt_dma=False):
        # one combined unit per et: the CAST is emitted right after its chain
        # so it lands early in the in-order vector queue (no convoy behind
        # norm-muls); consecutive units are spaced >=2 groups by the fq drain
        ot = out_t if out_t is not None else yT
        units = []
        for et in range(C // 128):
            def u(et=et):
                pool = pools[et % len(pools)]
                yps_w = pool.tile([128, 2, QB], F32, tag="sT", name="yps")
                yps_t = yps_w[:, 0, :]
                for j, kc in enumerate(kcs):
                    nc.tensor.matmul(
                        yps_t,
                        lhsT=wp_sb[:, kc, et * 128:(et + 1) * 128],
                        rhs=attT_sb[kc][:, sqb * QB:(sqb + 1) * QB],
                        start=(j == 0),
                        stop=(j == len(kcs) - 1),
                    )
                ys = ysp.tile([128, QB], F16, tag="ys", name="ys")
                if alt_copy and et % 2 == 1:
                    nc.scalar.activation(ys[:, :], yps_t, AF.Copy)
                else:
                    nc.vector.tensor_copy(ys[:, :], yps_t)
                eng = nc.scalar if (alt_dma and et % 2 == 1) else nc.sync
                eng.dma_start(
                    out=ot[et * 128:(et + 1) * 128, sqb * QB:(sqb + 1) * QB],
                    in_=ys[:, :],
                )
            units.append(u)
        return units

    # ---------------- attention piece emitters ----------------
    def emit_S(h, qb, grp, sT):
        ktile = kT_sb[h // 2]
        qtile = qT_sb[h // 2]
        po = (h % 2) * D
        diag = grp >= 2 * (qb + 1) - 2
        for m in range(2):
            jt = grp * 2 + m
            c0 = 128 * (jt - 4 * qb) if diag else 0
            nc.tensor.matmul(
                sT[:, m, c0:QB],
                lhsT=ktile[po:po + D, jt * 128:(jt + 1) * 128],
                rhs=qtile[po:po + D, qb * QB + c0:(qb + 1) * QB],
                start=True,
                stop=True,
            )

    def emit_exp(h, qb, grp, sT, pT):
        diag = grp >= 2 * (qb + 1) - 2
        if diag:
            # ONE fused exp for both m j-tiles starting at m0's causal
            # offset; the m1 columns below its own offset become junk in pT
            # but the O-matmul only streams pT from its per-tile offset, so
            # they are never read. Saves the ~293ns fixed ACTIVATE overhead
            # of a second instruction on the pacing ACT engine.
            c0 = 128 * (grp * 2 - 4 * qb)
            nc.scalar.activation(pT[:, :, c0:QB], sT[:, :, c0:QB], AF.Exp)
            # 0/1 causal mask on the diagonal 128-block, after exp so the
            # S->P critical path has no DVE hop; hidden by the O lag
            for m in range(2):
                cm = 128 * (grp * 2 + m - 4 * qb)
                nc.vector.tensor_mul(
                    pT[:, m, cm:cm + 128], pT[:, m, cm:cm + 128], mask_sb[:, :]
                )
        else:
            nc.scalar.activation(pT[:, :, :], sT[:, :, :], AF.Exp)

    def emit_O(h, qb, grp, pT, oT):
        njt = 4 * (qb + 1)
        diag = grp >= 2 * (qb + 1) - 2
        for m in range(2):
            jt = grp * 2 + m
            c0 = 128 * (jt - 4 * qb) if diag else 0
            nc.tensor.matmul(
                oT[:, c0:QB],
                lhsT=v_sb[jt][:, h, :],
                rhs=pT[:, m, c0:QB],
                start=(jt == 0),
                stop=(jt == njt - 1),
            )

    def emit_norm(h, qb, oT):
        po = (h % 2) * D
        rz = smallp.tile([D, QB], F32, tag="rz", name="rz")
        nc.vector.reciprocal_approx_fast(out=rz[:, :], in_=oT[0:D, :])
        nc.vector.tensor_mul(
            attT_sb[h // 2][po:po + D, qb * QB:(qb + 1) * QB],
            oT[D:2 * D, :],
            rz[:, :],
        )

    # ---------------- main schedule ----------------
    # chunk 0 upfront (the DMA ramp); spread its 4 psum tiles over both pools
    for un in qk_pair_units(0, (0, 2), widep):
        un()
    for un in qk_pair_units(0, (1, 3), widep):
        un()
    for un in v_pair_units(0, 0, fillp):
        un()
    for un in v_pair_units(0, 1, fillp):
        un()

    dq = []   # deadline units: chunk(qb+1) must finish within qb
    fq = []   # free units: stage6, drained evenly over ALL remaining groups

    total_groups = HPC * sum(2 * (qb + 1) for qb in range(NQB))
    state = {"tg_left": total_groups}

    def drain(qb_groups_left):
        n = -(-len(dq) // qb_groups_left) if qb_groups_left > 0 else len(dq)
        for _ in range(min(n, len(dq))):
            dq.pop(0)()
        nf = -(-len(fq) // state["tg_left"]) if state["tg_left"] > 0 else len(fq)
        for _ in range(min(nf, len(fq))):
            fq.pop(0)()

    for qb in range(NQB):
        ngr = 2 * (qb + 1)
        if qb >= 1:
            fq.extend(stage6_units(qb - 1, [fillp]))
        if qb < NQB - 1:
            dq.extend(chunk_units(qb + 1, fillp))
        g_left = HPC * ngr
        # heads run in pairs, groups round-robin across the pair: exp(h,g)
        # gets two heads' worth of PE time before its O-matmuls are due, so
        # the single ACT stream never gates the in-order PE
        for hp in ((0, 1), (2, 3)):
            oTs = {h: oTp.tile([128, QB], F32, tag="oT", name="oT") for h in hp}
            pend = []
            for grp in range(ngr):
                for h in hp:
                    sT = widep.tile([128, 2, QB], F32, tag="sT", name="sT")
                    emit_S(h, qb, grp, sT)
                    pT = pTp.tile([128, 2, QB], BF, tag="pT", name="pT")
                    emit_exp(h, qb, grp, sT, pT)
                    drain(g_left)
                    g_left -= 1
                    state["tg_left"] -= 1
                    pend.append((h, grp, pT))
                    if len(pend) > 4:
                        h0_, g0, p0 = pend.pop(0)
                        emit_O(h0_, qb, g0, p0, oTs[h0_])
            if dq:
                dq.pop(0)()
            elif fq:
                fq.pop(0)()
            for h0_, g0, p0 in pend:
                emit_O(h0_, qb, g0, p0, oTs[h0_])
            for h in hp:
                emit_norm(h, qb, oTs[h])
            if qb == NQB - 1 and hp == (0, 1):
                # last q-block: its kc0 out-proj half only needs heads 0,1 —
                # feed it to the tail-end groups as free filler
                fq.extend(stage6_units(NQB - 1, [fillp], kcs=(0,)))
        while dq:
            dq.pop(0)()
    while fq:
        fq.pop(0)()
    # tail: kc1 half of the last q-block -> yT2 (host-summed). A few junk
    # matmuls bridge the PE-idle norm window (keeps the HAM clock warm), then
    # the 8 units pipeline over two psum pools with alternating copy engines
    # and alternating DMA queues
    emit_warm(6, pool=widep)
    for un in stage6_units(NQB - 1, [fillp, widep], kcs=(1,), out_t=yT2,
                           alt_copy=True, alt_dma=True):
        un()


def build_nc():
    from contextlib import ExitStack

    nc = bacc.Bacc("TRN2", target_bir_lowering=False)
    xT = nc.dram_tensor("xT", [C, T], BF, kind="ExternalInput")
    wqk = nc.dram_tensor("wqk", [C, 2 * CS], BF, kind="ExternalInput")
    wv = nc.dram_tensor("wv", [C, CS], BF, kind="ExternalInput")
    bqk = nc.dram_tensor("bqk", [2 * CS, 1], F32, kind="ExternalInput")
    wp = nc.dram_tensor("wp", [CS, C], BF, kind="ExternalInput")
    masks = nc.dram_tensor("masks", [128, 128], BF, kind="ExternalInput")
    yT = nc.dram_tensor("yT", [C, T], F16, kind="ExternalOutput")
    yT2 = nc.dram_tensor("yT2", [C, T], F16, kind="ExternalOutput")
    with tile.TileContext(nc) as tc:
        with nc.allow_low_precision(reason="bf16 matmul inputs; accumulation stays fp32 in PSUM"):
            with ExitStack() as ctx:
                _build_body(nc, tc, ctx, xT, wqk, wv, bqk, wp, masks, yT, yT2)
    nc.compile()
    return nc


def make_mask01():
    r = np.arange(128)[:, None]
    c = np.arange(128)[None, :]
    return np.where(r <= c, 1.0, 0.0).astype(BF_NP)


def make_in_maps(x, W_qkv, b_qkv, W_proj):
    scale = np.float32(1.0 / np.sqrt(D))
    mask_h = make_mask01()
    in_maps = []
    for i in range(NCORES):
        b, g = divmod(i, HPC)
        cs0 = g * CS
        wq = W_qkv[:, cs0:cs0 + CS] * scale
        wk = W_qkv[:, C + cs0:C + cs0 + CS]
        bq = b_qkv[cs0:cs0 + CS] * scale
        bk = b_qkv[C + cs0:C + cs0 + CS]
        in_maps.append({
            "xT": np.ascontiguousarray(x[b].T).astype(BF_NP),
            "wqk": np.concatenate([wq, wk], axis=1).astype(BF_NP),
            "wv": np.ascontiguousarray(W_qkv[:, 2 * C + cs0:2 * C + cs0 + CS]).astype(BF_NP),
            "bqk": np.concatenate([bq, bk])[:, None].astype(np.float32),
            "wp": np.ascontiguousarray(W_proj[cs0:cs0 + CS, :]).astype(BF_NP),
            "masks": mask_h,
        })
    return in_maps


_NC_CACHE = None


def _get_nc():
    global _NC_CACHE
    if _NC_CACHE is None:
        _NC_CACHE = build_nc()
    return _NC_CACHE


def gather(results, b_qkv, W_proj, b_proj):
    Y = np.zeros((B, T, C), np.float32)
    for i in range(NCORES):
        Y[i // HPC] += results[i]["yT"].T.astype(np.float32)
        qb3 = (NQB - 1) * QB
        Y[i // HPC][qb3:] += results[i]["yT2"].T[qb3:].astype(np.float32)
    Y += (b_qkv[2 * C:].astype(np.float32) @ W_proj.astype(np.float32)
          + b_proj.astype(np.float32))[None, None, :]
    return Y


def kernel(x, W_qkv, b_qkv, W_proj, b_proj):
    global LAST_RESULT
    x = np.asarray(x, np.float32)
    W_qkv = np.asarray(W_qkv, np.float32)
    b_qkv = np.asarray(b_qkv, np.float32)
    W_proj = np.asarray(W_proj, np.float32)
    b_proj = np.asarray(b_proj, np.float32)

    nc = _get_nc()
    in_maps = make_in_maps(x, W_qkv, b_qkv, W_proj)
    res = run_bass_kernel_spmd(nc, in_maps, list(range(NCORES)), trace=TRACE)
    LAST_RESULT = res
    if TRACE and res.exec_time_ns is not None:
        print(f"HW exec time: {res.exec_time_ns} ns")
    return gather(res.results, b_qkv, W_proj, b_proj)
